# revision 1
# baseline (speedup 1.0000x reference)
"""Multi-head attention (B=4, S=2048, D=256, H=4) on 8 trn2 NeuronCores.

Sharding: core c handles batch b = c//2 and query half qh = c%2 (1024
queries), all 4 heads, full 2048 keys.  Inputs are pre-transposed on the
host (x[b].T and W.T) so every matmul contraction runs with the
contracted dim on SBUF partitions and no on-device transposes are
needed.

Per-core dataflow (scores kept transposed: [keys, queries]):
  QT = WQT.T-chunks @ xq          -> Q.T   [256(feat), 1024(q)]
  KT = WKT.T-chunks @ xT          -> K.T   [256(feat), 2048(k)]
  V  = xT-chunks.T  @ WVT         -> V_aug [2048(k), 4, 65]  (ones col)
  per head pair p, query half f, key tile kt (128 keys):
     S.T[kt, q] = KT_h-slices.T @ QT-slices  (2 heads row-packed in PE)
     E.T        = exp(S.T * scale + mask_bias[key])      (ScalarE)
     cd_h      += V_aug_h.T @ E.T   (rows 0-63 = ctx.T, row 64 = den;
                                     one PSUM bank per head, accumulated
                                     over the 16 key tiles)
  ctx normalized by 1/den (PE broadcast of reciprocal + DVE multiply)
  out = sum_h ctxn_h.T-chunks @ WOT_h  -> [1024(q), 256]

Matmul operands are float32r (TF32-like, 1 PE cycle/col for N>=256 vs 4
cycles for fp32's LOW_HIGH two-pass mode).  fp32r matmuls must write
PSUM at partition offset 0 (ISA rule s3d3_mm_valid_dst_partition), which
is why each head's ctx accumulator lives in its own bank instead of
being column-packed.  The reciprocal/broadcast path stays full fp32.
"""

import sys

for _p in ("/opt/trn_rl_repo",):
    if _p not in sys.path:
        sys.path.insert(0, _p)

import numpy as np

B, S, D, H, HD = 4, 2048, 256, 4, 64
SCALE = HD**-0.5
NCORES = 8
QS = S // 2  # queries per core
QH = QS // 2  # query half (one psum bank wide per head)
P = 128
NKT = S // P  # 16 key tiles

_cache = {}


def _build_nc():
    import concourse.mybir as mybir
    from concourse import bacc
    from concourse.tile import TileContext

    f32 = mybir.dt.float32
    f32r = mybir.dt.float32r
    Exp = mybir.ActivationFunctionType.Exp
    Ln = getattr(mybir.ActivationFunctionType, 'Ln', None) or mybir.ActivationFunctionType.Log

    nc = bacc.Bacc("TRN2", target_bir_lowering=False, debug=False)

    xT_d = nc.dram_tensor("xT", [D, S], f32, kind="ExternalInput")
    xq_d = nc.dram_tensor("xq", [D, QS], f32, kind="ExternalInput")
    wqt_d = nc.dram_tensor("wqt", [D, D], f32, kind="ExternalInput")
    wkt_d = nc.dram_tensor("wkt", [D, D], f32, kind="ExternalInput")
    wvt_d = nc.dram_tensor("wvt", [D, D], f32, kind="ExternalInput")
    wot_d = nc.dram_tensor("wot", [D, D], f32, kind="ExternalInput")
    bias_d = nc.dram_tensor("bias", [P, NKT], f32, kind="ExternalInput")
    out_d = nc.dram_tensor("out", [QS, D], f32, kind="ExternalOutput")

    with TileContext(nc) as tc:
        with (
            tc.tile_pool(name="const", bufs=1) as const,
            tc.tile_pool(name="big", bufs=1) as big,
            tc.tile_pool(name="et", bufs=6) as etp,
            tc.tile_pool(name="small", bufs=2) as small,
            tc.tile_pool(name="psA", bufs=2, space="PSUM") as psA,
            tc.tile_pool(name="psCD", bufs=2, space="PSUM") as psCD,
        ):
            # ---- constants / weights / bias ----
            ones4 = const.tile([P, 4], f32)
            nc.vector.memset(ones4, 1.0)
            ones_row_f = const.tile([65, P], f32)
            nc.vector.memset(ones_row_f, 1.0)
            ones_row = const.tile([65, P], f32r)
            nc.vector.tensor_copy(ones_row, ones_row_f)
            bias_sb = const.tile([P, NKT], f32)
            nc.gpsimd.dma_start(out=bias_sb, in_=bias_d[:, :])

            # spread input DMAs over four engine queues so the issues overlap;
            # Q-path inputs (wqt, xq) first so projections start early.
            w_sb = {}
            w_engines = {"wqt": nc.scalar, "wkt": nc.gpsimd, "wvt": nc.gpsimd}
            # (wqt then xq then xT all on the scalar HWDGE queue: the DMA
            # engines round-robin across queues, so sharing one queue is
            # the only way to prioritize the Q-path inputs)
            for nm, dram in (("wqt", wqt_d), ("wkt", wkt_d), ("wvt", wvt_d)):
                wt = const.tile([P, 2, D], f32r, name=f"w_{nm}", tag=f"w_{nm}")
                w_engines[nm].dma_start(
                    out=wt, in_=dram.rearrange("(c p) e -> p c e", p=P).bitcast(f32r)
                )
                w_sb[nm] = wt
            xq_sb = []
            for c in range(2):
                xq = big.tile([P, QS], f32r, name=f"xq{c}", tag=f"xq{c}")
                [nc.scalar, nc.scalar][c].dma_start(
                    out=xq, in_=xq_d[c * P : (c + 1) * P, :].bitcast(f32r)
                )
                xq_sb.append(xq)
            xT_sb = []
            for c in range(2):
                xt = big.tile([P, S], f32r, name=f"xT{c}", tag=f"xT{c}")
                nc.scalar.dma_start(out=xt, in_=xT_d[c * P : (c + 1) * P, :].bitcast(f32r))
                xT_sb.append(xt)
            # W_O.T grouped per head: [64, 4, 256] so each head's contraction
            # chunk starts at partition 0.
            wot_sb = const.tile([64, 4, D], f32r, name="w_wot", tag="w_wot")
            nc.gpsimd.dma_start(
                out=wot_sb, in_=wot_d.rearrange("(h p) e -> p h e", p=64).bitcast(f32r)
            )

            # ---- projections (emitted lazily so they interleave with
            # attention: the Tile scheduler + in-order engines execute
            # roughly in emission order, and the ScalarE-bound attention
            # steady state leaves PE gaps that this work fills, which also
            # keeps the PE's HAM clock-gate warm) ----
            QT_sb = [None, None]
            KT_sb = [None, None]
            V_sb = [None] * NKT
            ctxn_sb = []
            for h in range(H):
                cn = big.tile([64, QS], f32r, name=f"ctxn{h}", tag=f"ctxn{h}")
                ctxn_sb.append(cn)

            def qt_proj(m):
                qt = big.tile([P, QS], f32r, name=f"QT{m}", tag=f"QT{m}")
                ps = psA.tile([P, 1024], f32, name="psA", tag="psA")
                for n in range(QS // 512):
                    for c in range(2):
                        nc.tensor.matmul(
                            ps[:, n * 512 : (n + 1) * 512],
                            w_sb["wqt"][:, c, m * P : (m + 1) * P],
                            xq_sb[c][:, n * 512 : (n + 1) * 512],
                            start=(c == 0),
                            stop=(c == 1),
                        )
                nc.vector.tensor_copy(qt, ps)
                QT_sb[m] = qt

            def kt_proj(m, half):
                if KT_sb[m] is None:
                    KT_sb[m] = big.tile([P, S], f32r, name=f"KT{m}", tag=f"KT{m}")
                kt_t = KT_sb[m]
                ps = psA.tile([P, 1024], f32, name="psA", tag="psA")
                for n in range(2):
                    for c in range(2):
                        nc.tensor.matmul(
                            ps[:, n * 512 : (n + 1) * 512],
                            w_sb["wkt"][:, c, m * P : (m + 1) * P],
                            xT_sb[c][:, half * 1024 + n * 512 : half * 1024 + (n + 1) * 512],
                            start=(c == 0),
                            stop=(c == 1),
                        )
                nc.vector.tensor_copy(kt_t[:, half * 1024 : (half + 1) * 1024], ps)

            def v_proj(mt):
                # V_aug [s-tile, 4, 65]: per-head 64 value cols + a ones col
                # (whose cd-matmul row is the softmax denominator).
                vt = big.tile([P, 4, 65], f32r, name=f"V{mt}", tag=f"V{mt}")
                ps = psCD.tile([P, 512], f32, name="ps_v", tag="psCD")
                for c in range(2):
                    nc.tensor.matmul(
                        ps[:, :D],
                        xT_sb[c][:, mt * P : (mt + 1) * P],
                        w_sb["wvt"][:, c, :],
                        start=(c == 0),
                        stop=(c == 1),
                    )
                nc.vector.tensor_copy(
                    vt[:, :, 0:64], ps[:, :D].rearrange("p (h e) -> p h e", h=4)
                )
                nc.vector.tensor_copy(vt[:, :, 64], ones4)
                V_sb[mt] = vt

            def kt_loop(p, f, inject=None):
                # rows 0-63: ctx.T for head 2p+h2; row 64: denominator.
                # One bank per head (h2 chooses the 512-col half).
                ps_cd = psCD.tile([65, 1024], f32, name="ps_cd", tag="psCD")
                for kt in range(NKT):
                    ps_s = psA.tile([P, 1024], f32, name="ps_s", tag="psA")
                    # warming matmul: same operands as the h2=0 scores matmul,
                    # overwritten by it (start=True clears the bank).  Fills
                    # the PE's ScalarE-wait gaps so the HAM activity monitor
                    # keeps the PE clock at 2.4GHz instead of oscillating
                    # down to 1.2GHz (which would make PE the bottleneck).
                    nc.tensor.matmul(
                        ps_s[:, 0:QH],
                        KT_sb[p][0:64, kt * P : (kt + 1) * P],
                        QT_sb[p][0:64, f * QH : (f + 1) * QH],
                        start=True,
                        stop=True,
                        tile_position=(0, 0),
                    )
                    for h2 in range(2):
                        nc.tensor.matmul(
                            ps_s[:, h2 * 512 : h2 * 512 + QH],
                            KT_sb[p][64 * h2 : 64 * h2 + 64, kt * P : (kt + 1) * P],
                            QT_sb[p][64 * h2 : 64 * h2 + 64, f * QH : (f + 1) * QH],
                            start=True,
                            stop=True,
                            tile_position=(64 * h2, 0),
                        )
                    et = etp.tile([P, 1024], f32r, name="et", tag="et")
                    nc.scalar.activation(
                        et, ps_s, Exp, bias=bias_sb[:, kt : kt + 1], scale=SCALE
                    )
                    for h2 in range(2):
                        h = 2 * p + h2
                        nc.tensor.matmul(
                            ps_cd[0:65, h2 * 512 : h2 * 512 + QH],
                            V_sb[kt][:, h, :],
                            et[:, h2 * 512 : h2 * 512 + QH],
                            start=(kt == 0),
                            stop=(kt == NKT - 1),
                        )
                    if inject and kt in inject:
                        inject[kt]()
                return ps_cd

            def finish_cd(ps_cd):
                # Emitted right after a section's kt-loop: evict ctx+den to
                # SBUF and take the reciprocal of the den row (DVE, 6.5us for
                # a single-partition row, overlapped with the next section).
                # Releases the PSUM slot one section early.
                cdsb = small.tile([65, 1024], f32, name="cdsb", tag="cdsb")
                nc.vector.tensor_copy(cdsb, ps_cd)
                recip = small.tile([65, 1024], f32r, name="recip", tag="recip")
                with nc.allow_low_precision(reason="f32r rounding of 1/den"):
                    nc.vector.reciprocal(recip[64:65, :], ps_cd[64:65, :])
                return cdsb, recip

            def apply_norm(p, f, fin):
                # Emitted two sections later (so the reciprocal is long done
                # and the PE broadcast can't head-of-line-block anything):
                # PE row-broadcast of 1/den, then DVE multiplies write the
                # normalized ctx.T to its per-head SBUF tile.
                cdsb, recip = fin
                ps_r = psA.tile([P, 1024], f32, name="ps_r", tag="psA")
                for h2 in range(2):
                    nc.tensor.matmul(
                        ps_r[:, h2 * 512 : h2 * 512 + QH],
                        ones_row[64:65, :],
                        recip[64:65, h2 * 512 : h2 * 512 + QH],
                        start=True,
                        stop=True,
                        tile_position=(64, 0),
                    )
                r_sb = small.tile([P, 1024], f32, name="r_sb", tag="r_sb")
                nc.vector.tensor_copy(r_sb, ps_r)
                for h2 in range(2):
                    nc.vector.tensor_mul(
                        ctxn_sb[2 * p + h2][:, f * QH : (f + 1) * QH],
                        cdsb[0:64, h2 * 512 : h2 * 512 + QH],
                        r_sb[0:64, h2 * 512 : h2 * 512 + QH],
                    )

            def oproj(m):
                # contract over 4 per-head chunks of 64
                ps = psCD.tile([P, 512], f32, name="ps_o", tag="psCD")
                for h in range(H):
                    nc.tensor.matmul(
                        ps[:, :D],
                        ctxn_sb[h][:, m * P : (m + 1) * P],
                        wot_sb[:, h, :],
                        start=(h == 0),
                        stop=(h == H - 1),
                    )
                ot = small.tile([P, D], f32, name="ot", tag="ot")
                nc.vector.tensor_copy(ot, ps[:, :D])
                nc.sync.dma_start(out=out_d[m * P : (m + 1) * P, :], in_=ot)

            # prologue: only what the first section needs immediately
            qt_proj(0)
            kt_proj(0, 0)
            kt_proj(0, 1)
            for mt in range(3):
                v_proj(mt)

            # section (0,0): stream remaining V tiles 2 steps ahead of their
            # cd-use; pair-1 Q/K projections fill later steps.
            inj00 = {kt: (lambda mt=kt + 2: v_proj(mt)) for kt in range(1, NKT - 2)}
            inj00[NKT - 2] = lambda: qt_proj(1)
            inj00[NKT - 1] = lambda: kt_proj(1, 0)
            cd00 = kt_loop(0, 0, inj00)
            fin00 = finish_cd(cd00)
            cd10 = kt_loop(1, 0, {1: lambda: kt_proj(1, 1)})
            fin10 = finish_cd(cd10)
            cd01 = kt_loop(0, 1, {0: lambda: apply_norm(0, 0, fin00)})
            fin01 = finish_cd(cd01)
            cd11 = kt_loop(
                1,
                1,
                {
                    0: lambda: apply_norm(1, 0, fin10),
                    4: lambda: oproj(0),
                    6: lambda: oproj(1),
                    8: lambda: oproj(2),
                    10: lambda: oproj(3),
                },
            )
            fin11 = finish_cd(cd11)
            apply_norm(0, 1, fin01)
            apply_norm(1, 1, fin11)
            for m in range(4, 8):
                oproj(m)

    nc.compile()
    return nc


def _get_nc():
    if "nc" not in _cache:
        _cache["nc"] = _build_nc()
    return _cache["nc"]


def make_in_maps(x, W_Q, W_K, W_V, W_O, mask):
    wqt = np.ascontiguousarray(W_Q.T).astype(np.float32)
    wkt = np.ascontiguousarray(W_K.T).astype(np.float32)
    wvt = np.ascontiguousarray(W_V.T).astype(np.float32)
    wot = np.ascontiguousarray(W_O.T).astype(np.float32)
    in_maps = []
    for c in range(NCORES):
        b, qh = c // 2, c % 2
        xT_b = np.ascontiguousarray(np.asarray(x[b]).T).astype(np.float32)
        xq = np.ascontiguousarray(xT_b[:, qh * QS : (qh + 1) * QS])
        bias = np.where(np.asarray(mask[b]) == 0, -1e30, 0.0).astype(np.float32)
        bias = np.ascontiguousarray(bias.reshape(NKT, P).T)
        in_maps.append(
            {
                "xT": xT_b,
                "xq": xq,
                "wqt": wqt,
                "wkt": wkt,
                "wvt": wvt,
                "wot": wot,
                "bias": bias,
            }
        )
    return in_maps


def gather(results):
    out = np.empty((B, S, D), np.float32)
    for c in range(NCORES):
        b, qh = c // 2, c % 2
        out[b, qh * QS : (qh + 1) * QS, :] = results[c]["out"]
    return out


def kernel(x, W_Q, W_K, W_V, W_O, mask):
    from concourse.bass_utils import run_bass_kernel_spmd

    nc = _get_nc()
    in_maps = make_in_maps(x, W_Q, W_K, W_V, W_O, mask)
    res = run_bass_kernel_spmd(nc, in_maps, core_ids=list(range(NCORES)))
    return gather(res.results)



# revision 8
# speedup vs baseline: 1.5223x; 1.5223x over previous
"""Multi-head attention (B=4, S=2048, D=256, H=4) on 8 trn2 NeuronCores.

Sharding: core c handles batch b = c//2 and query half qh = c%2 (1024
queries), all 4 heads, full 2048 keys.  The host rolls x[b].T by
-qh*1024 columns so every core's queries sit at columns 0-1023 of its
xT input (key order is free: softmax+sum over keys is permutation
invariant as long as the mask bias is rolled identically).  This keeps
the SPMD program core-agnostic and avoids a separate xq input.

Per-core dataflow (scores kept transposed: [keys, queries]):
  QT = WQT.T-chunks @ xT[:, :1024]  -> Q.T [256(feat), 1024(q)]
  KT = WKT.T-chunks @ xT            -> K.T [256(feat), 2048(k)]
  V  = xT-chunks.T  @ WVT           -> V_aug [2048(k), 4, 65] (ones col)
  per head pair p, query half f, key tile kt (128 keys):
     S.T[kt, q] = KT_h-slices.T @ QT-slices  (2 heads row-packed in PE)
     E.T        = exp(S.T * scale + mask_bias[key])      (ScalarE)
     cd_h      += V_aug_h.T @ E.T   (rows 0-63 = ctx.T, row 64 = den;
                                     one PSUM bank per head, accumulated
                                     over the 16 key tiles)
  after each section: cd evicted to SBUF; den row transposed to
  partitions via 8 single-row matmuls -> [128,8] -> DVE reciprocal.
  out tile m: per-head matmuls ps4[:,h,:] = ctx_h.T-chunk @ WOT_h, then
  out = sum_h ps4[:,h,:] * (1/den_h) via DVE per-partition-scalar
  multiply-adds (normalization folded into the output combine, which is
  valid per head since each head's 1/den[h,q] scales output rows).

The single-partition [1,1024] DVE reciprocal of the previous design
(6.5us, head-of-line blocking the in-order DVE FIFO and stalling the PE
long enough to re-trigger the HAM clock throttle) is gone; all
reciprocals run on [128,8] tiles.

Matmul operands are float32r (TF32-like, 1 PE cycle/col for N>=256).
fp32r matmuls must write PSUM at partition offset 0, which all dsts
here do.  Input DMAs are split into 512-col pieces issued on the sync
and gpsimd queues in consumption order so projections and the first
attention section start while the bulk of xT is still in flight.
"""

import sys

for _p in ("/opt/trn_rl_repo",):
    if _p not in sys.path:
        sys.path.insert(0, _p)

import numpy as np

B, S, D, H, HD = 4, 2048, 256, 4, 64
SCALE = HD**-0.5
NCORES = 8
QS = S // 2  # queries per core
QH = QS // 2  # query half (one psum bank wide per head)
P = 128
NKT = S // P  # 16 key tiles

_cache = {}


def _build_nc():
    import concourse.mybir as mybir
    from concourse import bacc
    from concourse.tile import TileContext

    f32 = mybir.dt.float32
    f32r = mybir.dt.float32r
    Exp = mybir.ActivationFunctionType.Exp
    Alu = mybir.AluOpType

    nc = bacc.Bacc("TRN2", target_bir_lowering=False, debug=False)

    xT_d = nc.dram_tensor("xT", [D, S], f32, kind="ExternalInput")
    wqt_d = nc.dram_tensor("wqt", [D, D], f32, kind="ExternalInput")
    wkt_d = nc.dram_tensor("wkt", [D, D], f32, kind="ExternalInput")
    wvt_d = nc.dram_tensor("wvt", [D, D], f32, kind="ExternalInput")
    wot_d = nc.dram_tensor("wot", [D, D], f32, kind="ExternalInput")
    bias_d = nc.dram_tensor("bias", [P, NKT], f32, kind="ExternalInput")
    out_d = nc.dram_tensor("out", [QS, D], f32, kind="ExternalOutput")

    with TileContext(nc) as tc:
        with (
            tc.tile_pool(name="const", bufs=1) as const,
            tc.tile_pool(name="big", bufs=1) as big,
            tc.tile_pool(name="et", bufs=6) as etp,
            tc.tile_pool(name="small", bufs=2) as small,
            tc.tile_pool(name="psA", bufs=2, space="PSUM") as psA,
            tc.tile_pool(name="psCD", bufs=2, space="PSUM") as psCD,
        ):
            # ---- input DMAs, split by first consumption and ordered by
            # priority on two hardware queues (sync: Q/K path, gpsimd: V/
            # later xT pieces/O path).  The scalar engine issues nothing so
            # the exp ACTIVATEs never queue behind a DMA descriptor gen. ----
            w_sb = {}
            for nm, dram, eng in (
                ("wqt", wqt_d, nc.sync),
                ("wkt", wkt_d, nc.sync),
                ("wvt", wvt_d, nc.gpsimd),
            ):
                wt = const.tile([P, 2, D], f32r, name=f"w_{nm}", tag=f"w_{nm}")
                eng.dma_start(
                    out=wt, in_=dram.rearrange("(c p) e -> p c e", p=P).bitcast(f32r)
                )
                w_sb[nm] = wt
            bias_sb = const.tile([P, NKT], f32)
            nc.gpsimd.dma_start(out=bias_sb, in_=bias_d[:, :])

            xT_sb = []
            for c in range(2):
                xt = big.tile([P, S], f32r, name=f"xT{c}", tag=f"xT{c}")
                xT_sb.append(xt)
            # pieces in consumption order: q0 feeds Q-proj(f=0) + K tiles
            # 0-3 + V tiles 0-3; q1 feeds Q-proj(f=1) + K/V tiles 4-7; ...
            for q, eng in ((0, nc.sync), (1, nc.sync), (2, nc.gpsimd), (3, nc.gpsimd)):
                for c in range(2):
                    eng.dma_start(
                        out=xT_sb[c][:, q * 512 : (q + 1) * 512],
                        in_=xT_d[c * P : (c + 1) * P, q * 512 : (q + 1) * 512].bitcast(
                            f32r
                        ),
                    )
            # W_O.T grouped per head: [64, 4, 256] so each head's contraction
            # chunk starts at partition 0.
            wot_sb = const.tile([64, 4, D], f32r, name="w_wot", tag="w_wot")
            nc.gpsimd.dma_start(
                out=wot_sb, in_=wot_d.rearrange("(h p) e -> p h e", p=64).bitcast(f32r)
            )

            # ---- constants ----
            ones4 = const.tile([P, 4], f32)
            nc.vector.memset(ones4, 1.0)
            one1 = const.tile([P, 2], f32r)
            nc.vector.tensor_copy(one1, ones4[:, 0:2])

            QT_sb = [None, None]
            KT_sb = [None, None]
            V_sb = [None] * NKT
            cd_sb = {}
            r_sb = {}
            for p in range(2):
                for f in range(2):
                    r_sb[(p, f)] = big.tile(
                        [P, 16], f32, name=f"r{p}{f}", tag=f"r{p}{f}"
                    )

            def qt_proj(m, n):
                # QT_sb[m][:, n*512:(n+1)*512] (feature rows m*128..)
                if QT_sb[m] is None:
                    QT_sb[m] = big.tile([P, QS], f32r, name=f"QT{m}", tag=f"QT{m}")
                ps = psA.tile([P, 512], f32, name="ps_q", tag="psA")
                for c in range(2):
                    nc.tensor.matmul(
                        ps[:, :],
                        w_sb["wqt"][:, c, m * P : (m + 1) * P],
                        xT_sb[c][:, n * 512 : (n + 1) * 512],
                        start=(c == 0),
                        stop=(c == 1),
                    )
                nc.vector.tensor_copy(QT_sb[m][:, n * 512 : (n + 1) * 512], ps)

            def kt_proj(m, q):
                # KT_sb[m][:, q*512:(q+1)*512]
                if KT_sb[m] is None:
                    KT_sb[m] = big.tile([P, S], f32r, name=f"KT{m}", tag=f"KT{m}")
                ps = psA.tile([P, 512], f32, name="ps_k", tag="psA")
                for c in range(2):
                    nc.tensor.matmul(
                        ps[:, :],
                        w_sb["wkt"][:, c, m * P : (m + 1) * P],
                        xT_sb[c][:, q * 512 : (q + 1) * 512],
                        start=(c == 0),
                        stop=(c == 1),
                    )
                nc.vector.tensor_copy(KT_sb[m][:, q * 512 : (q + 1) * 512], ps)

            def v_proj(mt):
                # V_aug [s-tile, 4, 65]: per-head 64 value cols + a ones col
                # (whose cd-matmul row is the softmax denominator).
                vt = big.tile([P, 4, 65], f32r, name=f"V{mt}", tag=f"V{mt}")
                ps = psCD.tile([P, 512], f32, name="ps_v", tag="psCD")
                for c in range(2):
                    nc.tensor.matmul(
                        ps[:, :D],
                        xT_sb[c][:, mt * P : (mt + 1) * P],
                        w_sb["wvt"][:, c, :],
                        start=(c == 0),
                        stop=(c == 1),
                    )
                nc.vector.tensor_copy(
                    vt[:, :, 0:64], ps[:, :D].rearrange("p (h e) -> p h e", h=4)
                )
                nc.vector.tensor_copy(vt[:, :, 64], ones4)
                V_sb[mt] = vt

            def kt_loop(p, f, inject=None):
                # rows 0-63: ctx.T for head 2p+h2; row 64: denominator.
                # One bank per head (h2 chooses the 512-col half).
                ps_cd = psCD.tile([65, 1024], f32, name="ps_cd", tag="psCD")
                for kt in range(NKT):
                    ps_s = psA.tile([P, 1024], f32, name="ps_s", tag="psA")
                    for h2 in range(2):
                        nc.tensor.matmul(
                            ps_s[:, h2 * 512 : h2 * 512 + QH],
                            KT_sb[p][64 * h2 : 64 * h2 + 64, kt * P : (kt + 1) * P],
                            QT_sb[p][64 * h2 : 64 * h2 + 64, f * QH : (f + 1) * QH],
                            start=True,
                            stop=True,
                            tile_position=(64 * h2, 0),
                        )
                    et = etp.tile([P, 1024], f32r, name="et", tag="et")
                    nc.scalar.activation(
                        et, ps_s, Exp, bias=bias_sb[:, kt : kt + 1], scale=SCALE
                    )
                    for h2 in range(2):
                        h = 2 * p + h2
                        nc.tensor.matmul(
                            ps_cd[0:65, h2 * 512 : h2 * 512 + QH],
                            V_sb[kt][:, h, :],
                            et[:, h2 * 512 : h2 * 512 + QH],
                            start=(kt == 0),
                            stop=(kt == NKT - 1),
                        )
                    if inject and kt in inject:
                        inject[kt]()
                return ps_cd

            def finish_cd(p, f, ps_cd):
                # Evict ctx+den to SBUF right at section end, freeing the
                # PSUM slot for the next-but-one section.
                cdsb = big.tile([65, 1024], f32r, name=f"cd{p}{f}", tag=f"cd{p}{f}")
                nc.vector.tensor_copy(cdsb, ps_cd)
                cd_sb[(p, f)] = cdsb

            def den_recip(p, f):
                # Transpose the [1,1024] den row into partitions via 8
                # single-row matmuls, then one cheap [128,8] reciprocal.
                # col layout: h2*4 + q128 (q128 = 128-query block in half f).
                cdsb = cd_sb[(p, f)]
                ps_den = psCD.tile([P, 16], f32, name="ps_den", tag="psCD")
                for t in range(8):
                    h2, qq = t // 4, t % 4
                    # 2 duplicate output cols: fp32r ISA needs even free counts
                    nc.tensor.matmul(
                        ps_den[:, 2 * t : 2 * t + 2],
                        cdsb[64:65, h2 * 512 + qq * P : h2 * 512 + (qq + 1) * P],
                        one1[64:65, 0:2],
                        start=True,
                        stop=True,
                    )
                nc.vector.reciprocal(r_sb[(p, f)][:, 0:16], ps_den[:, 0:16])

            def oproj(m):
                # out tile m (queries m*128..): per-head matmul (no accum
                # across heads), then normalization folded into the combine:
                # out = sum_h ps4[:,h,:] * (1/den_h) with per-partition
                # scalars from r_sb.
                f, qq = m // 4, m % 4
                ps4 = psCD.tile([P, 4, D], f32, name="ps4", tag="psCD")
                for h in range(H):
                    p, h2 = h // 2, h % 2
                    nc.tensor.matmul(
                        ps4[:, h, :],
                        cd_sb[(p, f)][0:64, h2 * 512 + qq * P : h2 * 512 + (qq + 1) * P],
                        wot_sb[:, h, :],
                        start=True,
                        stop=True,
                    )

                def r(h):
                    c = 2 * ((h % 2) * 4 + qq)
                    return r_sb[(h // 2, f)][:, c : c + 1]

                acc = small.tile([P, D], f32, name="acc", tag="acc")
                nc.vector.tensor_scalar_mul(acc, ps4[:, 0, :], r(0))
                for h in range(1, H):
                    dst = (
                        small.tile([P, D], f32, name="acc", tag="acc")
                        if h < H - 1
                        else small.tile([P, D], f32, name="ot", tag="ot", bufs=3)
                    )
                    nc.vector.scalar_tensor_tensor(
                        dst, ps4[:, h, :], r(h), acc, Alu.mult, Alu.add
                    )
                    acc = dst
                nc.sync.dma_start(out=out_d[m * P : (m + 1) * P, :], in_=acc)

            # ---- prologue: only what section (0,0) needs immediately ----
            qt_proj(0, 0)
            kt_proj(0, 0)
            for mt in range(3):
                v_proj(mt)

            # section (0,0): stream V tiles 2 steps ahead of their cd-use;
            # remaining K columns and pair-1 projections fill later steps.
            inj00 = {
                1: lambda: v_proj(3),
                2: lambda: (v_proj(4), kt_proj(0, 1)),
                3: lambda: v_proj(5),
                4: lambda: v_proj(6),
                5: lambda: (v_proj(7), kt_proj(0, 2)),
                6: lambda: v_proj(8),
                7: lambda: v_proj(9),
                8: lambda: (v_proj(10), kt_proj(0, 3)),
                9: lambda: v_proj(11),
                10: lambda: v_proj(12),
                11: lambda: (v_proj(13), qt_proj(1, 0)),
                12: lambda: (v_proj(14), kt_proj(1, 0)),
                13: lambda: (v_proj(15), kt_proj(1, 1)),
                14: lambda: kt_proj(1, 2),
                15: lambda: kt_proj(1, 3),
            }
            cd00 = kt_loop(0, 0, inj00)
            finish_cd(0, 0, cd00)
            cd10 = kt_loop(
                1, 0, {1: lambda: den_recip(0, 0), 3: lambda: qt_proj(0, 1)}
            )
            finish_cd(1, 0, cd10)
            cd01 = kt_loop(
                0, 1, {1: lambda: den_recip(1, 0), 3: lambda: qt_proj(1, 1)}
            )
            finish_cd(0, 1, cd01)
            cd11 = kt_loop(
                1,
                1,
                {
                    1: lambda: den_recip(0, 1),
                    4: lambda: oproj(0),
                    6: lambda: oproj(1),
                    8: lambda: oproj(2),
                    10: lambda: oproj(3),
                },
            )
            finish_cd(1, 1, cd11)
            den_recip(1, 1)
            for m in range(4, 8):
                oproj(m)

    nc.compile()
    return nc


def _get_nc():
    if "nc" not in _cache:
        _cache["nc"] = _build_nc()
    return _cache["nc"]


def make_in_maps(x, W_Q, W_K, W_V, W_O, mask):
    wqt = np.ascontiguousarray(W_Q.T).astype(np.float32)
    wkt = np.ascontiguousarray(W_K.T).astype(np.float32)
    wvt = np.ascontiguousarray(W_V.T).astype(np.float32)
    wot = np.ascontiguousarray(W_O.T).astype(np.float32)
    in_maps = []
    for c in range(NCORES):
        b, qh = c // 2, c % 2
        xT_b = np.asarray(x[b]).T.astype(np.float32)
        xT_roll = np.ascontiguousarray(np.roll(xT_b, -qh * QS, axis=1))
        bias = np.where(np.asarray(mask[b]) == 0, -1e30, 0.0).astype(np.float32)
        bias = np.roll(bias, -qh * QS)
        bias = np.ascontiguousarray(bias.reshape(NKT, P).T)
        in_maps.append(
            {
                "xT": xT_roll,
                "wqt": wqt,
                "wkt": wkt,
                "wvt": wvt,
                "wot": wot,
                "bias": bias,
            }
        )
    return in_maps


def gather(results):
    out = np.empty((B, S, D), np.float32)
    for c in range(NCORES):
        b, qh = c // 2, c % 2
        out[b, qh * QS : (qh + 1) * QS, :] = results[c]["out"]
    return out


def kernel(x, W_Q, W_K, W_V, W_O, mask):
    from concourse.bass_utils import run_bass_kernel_spmd

    nc = _get_nc()
    in_maps = make_in_maps(x, W_Q, W_K, W_V, W_O, mask)
    res = run_bass_kernel_spmd(nc, in_maps, core_ids=list(range(NCORES)))
    return gather(res.results)
